# revision 6
# baseline (speedup 1.0000x reference)
"""GRU decoder with attention — Trainium2 Bass kernel, 8-way tensor-parallel.

Strategy (memory-bound: ~760 MiB of weights/activations read once):
 - All big GEMMs reassociated into GEMVs:  att = enc @ (Wk.T @ q),
   ctx = enc.T @ alphas, c = Wv @ ctx.
 - "Hostile" GEMVs (contract over the stored row's free dim) run on the
   Vector engine via fused scalar_tensor_tensor + accum_out (one pass,
   ~478 GB/s/core), weights output-row-sharded 8 ways (per-gate for GRU).
 - "Native" GEMVs (contract over stored rows) run on the PE with the
   vector as the 1-column stationary operand.
 - Cross-core exchange via small AllReduce/AllGather collectives
   (kq, att, ctx, c, h per layer, pre2) through Local DRAM bounce buffers.
"""

import numpy as np

H = 2048
V = 32
L = 5
T = 8192
NC = 8
SH = H // NC          # 256  row shard
TSH = T // NC         # 1024 encoder shard
CH = 2048             # chunk width for streaming tiles
C2H = 2 * H
SQRT_2_OVER_PI = 0.7978845608028654

_CACHE = {}

W_BUFS = 8
XB_BUFS = 5


def _build():
    import concourse.bacc as bacc
    import concourse.mybir as mybir
    import concourse.tile as tile
    from concourse.masks import make_identity

    FP32 = mybir.dt.float32
    A = mybir.AluOpType
    AF = mybir.ActivationFunctionType

    nc = bacc.Bacc("TRN2", target_bir_lowering=False, debug=False, num_devices=NC)

    def din(name, shape):
        return nc.dram_tensor(name, shape, FP32, kind="ExternalInput").ap()

    ENC = din("enc", [TSH, C2H])
    WQ = din("wq", [SH, H])
    WK = din("wk", [SH, C2H])
    WV = din("wv", [SH, C2H])
    WIH0 = din("wih0", [3, SH, C2H])
    WIHR = din("wihr", [L - 1, 3, SH, H])
    WHH = din("whh", [L, 3, SH, H])
    BIH = din("bih", [L, 3, SH])
    BHH = din("bhh", [L, 3, SH])
    BQ = din("bq", [SH])
    BV = din("bv", [SH])
    W1 = din("w1", [128, 3 * H])
    B1 = din("b1", [128])
    W2 = din("w2", [V, H // 2])
    B2 = din("b2", [V])
    HID = din("hid", [L, H])
    HIDSH = din("hidsh", [L, SH])
    TOK = din("tok", [H])
    GH = din("g_hid", [H])
    BH = din("beta_hid", [H])
    GO = din("g_out", [3 * H])
    BO = din("beta_out", [3 * H])
    SEL = din("selmat", [64, NC])

    LOGITS = nc.dram_tensor("logits", [V], FP32, kind="ExternalOutput").ap()
    HOUT = nc.dram_tensor("h_out", [L, 1, H], FP32, kind="ExternalOutput").ap()

    with tile.TileContext(nc) as tc:
        with (
            tc.tile_pool(name="consts", bufs=1) as consts,
            tc.tile_pool(name="ws", bufs=W_BUFS) as ws,
            tc.tile_pool(name="xbp", bufs=XB_BUFS) as xbp,
            tc.tile_pool(name="scrp", bufs=2) as scrp,
            tc.tile_pool(name="small", bufs=1) as sm,
            tc.tile_pool(name="psbc", bufs=2, space="PSUM") as psbc,
            tc.tile_pool(name="pstr", bufs=2, space="PSUM") as pstr,
            tc.tile_pool(name="psrow", bufs=4, space="PSUM") as psrow,
            tc.tile_pool(name="dram", bufs=1, space="DRAM") as dram,
        ):
            ones = consts.tile([1, 128], FP32, tag="ones")
            nc.any.memset(ones[:], 1.0)
            onesp = consts.tile([128, 1], FP32, tag="onesp")
            nc.any.memset(onesp[:], 1.0)
            ident = consts.tile([128, 128], FP32, tag="ident")
            make_identity(nc, ident[:])
            epsc = consts.tile([1, 1], FP32, tag="epsc")
            nc.any.memset(epsc[:], 1e-5)

            # ---------- helpers ----------
            def load_row(vec_ap, width, nm):
                """DRAM vector [width] -> list of sbuf [1, CH] chunk tiles."""
                chunks = []
                for c0 in range(0, width, CH):
                    w = min(CH, width - c0)
                    t = sm.tile([1, CH], FP32, tag="rowb", bufs=3,
                                name=f"row_{nm}_{c0}")
                    nc.gpsimd.dma_start(t[:, :w], vec_ap[None, c0 : c0 + w])
                    chunks.append(t)
                return chunks

            def bcast(row_chunks, width, nm):
                """list of [1, CH] chunks -> list of [128, CH] tiles."""
                tiles = []
                for ci, c0 in enumerate(range(0, width, CH)):
                    w = min(CH, width - c0)
                    row = row_chunks[ci]
                    t = xbp.tile([128, CH], FP32, tag="xb", name=f"xb_{nm}_{c0}")
                    for j in range(0, w, 512):
                        pb = psbc.tile([128, 512], FP32, tag="bc", name=f"pb_{nm}")
                        nc.tensor.matmul(
                            pb[:], ones[:], row[:, j : j + 512],
                            start=True, stop=True,
                        )
                        nc.scalar.copy(t[:, j : j + 512], pb[:])
                    tiles.append(t)
                return tiles

            def to_row(col, n, nm):
                """col: sbuf [p, n] -> sbuf [n, p]."""
                p = col.shape[0]
                pt = pstr.tile([n, p], FP32, tag="tr", name=f"tr_{nm}")
                nc.tensor.transpose(pt[:], col, ident[:p, :p])
                r = sm.tile([n, p], FP32, tag=f"row_{nm}", name=f"row_{nm}")
                nc.scalar.copy(r[:], pt[:])
                return r

            def to_col(row, n, m, nm):
                """row: sbuf [n, m] (m<=128) -> sbuf [m, n]."""
                pt = pstr.tile([m, n], FP32, tag="tr", name=f"trc_{nm}")
                nc.tensor.transpose(pt[:], row, ident[:n, :n])
                c = sm.tile([m, n], FP32, tag=f"col_{nm}", name=f"col_{nm}")
                nc.scalar.copy(c[:], pt[:])
                return c

            def stt_gemv(wt, xbt, acc_col, nm):
                scr = scrp.tile([128, CH], FP32, tag="scr", name=f"scr_{nm}")
                nc.vector.scalar_tensor_tensor(
                    out=scr[:, : wt.shape[1]], in0=wt, scalar=1.0, in1=xbt,
                    op0=A.mult, op1=A.mult, accum_out=acc_col,
                )

            def pe_bcast_scalar(src11, n, nm):
                """[1,1] -> sbuf [n, 1]"""
                pb = pstr.tile([n, 1], FP32, tag="tr", name=f"pbs_{nm}")
                nc.tensor.matmul(pb[:], ones[:, :n], src11, start=True, stop=True)
                o = sm.tile([n, 1], FP32, tag=f"bs_{nm}", name=f"bs_{nm}")
                nc.scalar.copy(o[:], pb[:])
                return o

            # ---------- setup: small transposed vectors ----------
            hpr = sm.tile([10, 128], FP32, tag="hpr")
            nc.gpsimd.dma_start(hpr[:], HIDSH.rearrange("l (a b) -> (l a) b", b=128))
            hpT = to_col(hpr[:], 10, 128, "hp")  # [128, 10] cols l*2+a

            bihr = sm.tile([30, 128], FP32, tag="bihr")
            nc.gpsimd.dma_start(bihr[:], BIH.rearrange("l g (a b) -> (l g a) b", b=128))
            bihT = to_col(bihr[:], 30, 128, "bih")  # [128, 30]
            bhhr = sm.tile([30, 128], FP32, tag="bhhr")
            nc.gpsimd.dma_start(bhhr[:], BHH.rearrange("l g (a b) -> (l g a) b", b=128))
            bhhT = to_col(bhhr[:], 30, 128, "bhh")
            bsumT = sm.tile([128, 30], FP32, tag="bsumT")
            nc.vector.tensor_tensor(bsumT[:], bihT[:], bhhT[:], A.add)

            bqr = sm.tile([2, 128], FP32, tag="bqr")
            nc.gpsimd.dma_start(bqr[:], BQ.rearrange("(a b) -> a b", b=128))
            bqT = to_col(bqr[:], 2, 128, "bq")  # [128, 2]
            bvr = sm.tile([2, 128], FP32, tag="bvr")
            nc.gpsimd.dma_start(bvr[:], BV.rearrange("(a b) -> a b", b=128))
            bvT = to_col(bvr[:], 2, 128, "bv")
            b1r = sm.tile([1, 128], FP32, tag="b1r")
            nc.gpsimd.dma_start(b1r[:], B1.rearrange("(a b) -> a b", b=128))
            b1T = to_col(b1r[:], 1, 128, "b1")  # [128, 1]
            b2r = sm.tile([1, V], FP32, tag="b2r")
            nc.gpsimd.dma_start(b2r[:], B2[None, :])
            b2T = to_col(b2r[:], 1, V, "b2")  # [32, 1]

            # c init = bv + s_t  (s_t = hidden[-1] shard)
            cinit = sm.tile([128, 2], FP32, tag="cinit")
            nc.vector.tensor_tensor(cinit[:], bvT[:], hpT[:, 8:10], A.add)

            # ---------- q = Wq @ s_t + bq ----------
            st_row = load_row(HID[L - 1], H, "st")
            stb = bcast(st_row, H, "st")
            accq = sm.tile([128, 2], FP32, tag="accq")
            for a in range(2):
                wt = ws.tile([128, CH], FP32, tag="w", name="wq_t")
                nc.sync.dma_start(wt[:], WQ[a * 128 : (a + 1) * 128, :])
                stt_gemv(wt[:], stb[0][:], accq[:, a : a + 1], f"q{a}")
            qv = sm.tile([128, 2], FP32, tag="qv")
            nc.vector.tensor_tensor(qv[:], accq[:], bqT[:], A.add)

            # ---------- kq = Wk.T @ q  (PE native) ----------
            kq_row = sm.tile([1, C2H], FP32, tag="kqrow")
            for half in range(2):
                wk0 = ws.tile([128, CH], FP32, tag="w", name="wk0")
                nc.sync.dma_start(wk0[:], WK[0:128, half * CH : (half + 1) * CH])
                wk1 = ws.tile([128, CH], FP32, tag="w", name="wk1")
                nc.sync.dma_start(wk1[:], WK[128:256, half * CH : (half + 1) * CH])
                wkt = [wk0, wk1]
                for cq in range(4):
                    pr = psrow.tile([1, 512], FP32, tag="rowp", name=f"kqp{half}{cq}")
                    for b in range(2):
                        nc.tensor.matmul(
                            pr[:], qv[:, b : b + 1],
                            wkt[b][:, cq * 512 : (cq + 1) * 512],
                            start=(b == 0), stop=(b == 1),
                        )
                    nc.scalar.copy(
                        kq_row[:, half * CH + cq * 512 : half * CH + (cq + 1) * 512],
                        pr[:],
                    )
            cc_kq_in = dram.tile([C2H], FP32, tag="cckqi")
            cc_kq_out = dram.tile([C2H], FP32, tag="cckqo")
            nc.gpsimd.dma_start(cc_kq_in[None, :], kq_row[:])
            nc.gpsimd.collective_compute(
                "AllReduce", A.add, replica_groups=[list(range(NC))],
                ins=[cc_kq_in[:].opt()], outs=[cc_kq_out[:].opt()],
            )
            kqb = bcast(load_row(cc_kq_out[:], C2H, "kq"), C2H, "kq")

            # ---------- att = enc_shard @ kq ----------
            att_acc = sm.tile([128, 16], FP32, tag="attacc")
            for b in range(8):
                for half in range(2):
                    ent = ws.tile([128, CH], FP32, tag="w", name="enc1t")
                    nc.sync.dma_start(
                        ent[:],
                        ENC[b * 128 : (b + 1) * 128, half * CH : (half + 1) * CH],
                    )
                    stt_gemv(
                        ent[:], kqb[half][:],
                        att_acc[:, half * 8 + b : half * 8 + b + 1], f"att{b}_{half}",
                    )
            att8 = sm.tile([128, 8], FP32, tag="att8")
            nc.vector.tensor_tensor(att8[:], att_acc[:, :8], att_acc[:, 8:], A.add)
            att_row = to_row(att8[:], 8, "att")
            cc_att_in = dram.tile([TSH], FP32, tag="ccatti")
            cc_att_out = dram.tile([T], FP32, tag="ccatto")
            nc.gpsimd.dma_start(cc_att_in[:].rearrange("(a b) -> a b", a=8), att_row[:])
            nc.gpsimd.collective_compute(
                "AllGather", A.bypass, replica_groups=[list(range(NC))],
                ins=[cc_att_in[:].opt()], outs=[cc_att_out[:].opt()],
            )

            # ---------- softmax ----------
            attf = sm.tile([128, 64], FP32, tag="attf")
            nc.gpsimd.dma_start(attf[:], cc_att_out[:].rearrange("(p j) -> p j", p=128))
            mx = sm.tile([128, 1], FP32, tag="mx")
            nc.vector.tensor_reduce(mx[:], attf[:], mybir.AxisListType.X, A.max)
            mrow = to_row(mx[:], 1, "mx")
            m1 = sm.tile([1, 1], FP32, tag="m1")
            nc.vector.tensor_reduce(m1[:], mrow[:], mybir.AxisListType.X, A.max)
            negm = sm.tile([1, 1], FP32, tag="negm")
            nc.scalar.mul(negm[:], m1[:], -1.0)
            negmb = pe_bcast_scalar(negm[:], 128, "negm")
            ex = sm.tile([128, 64], FP32, tag="ex")
            es = sm.tile([128, 1], FP32, tag="es")
            nc.scalar.activation(ex[:], attf[:], AF.Exp, bias=negmb[:], accum_out=es[:])
            erow = to_row(es[:], 1, "es")
            edum = sm.tile([1, 128], FP32, tag="edum")
            stot = sm.tile([1, 1], FP32, tag="stot")
            nc.scalar.activation(edum[:], erow[:], AF.Copy, accum_out=stot[:])
            invs = sm.tile([1, 1], FP32, tag="invs")
            nc.vector.reciprocal(invs[:], stot[:])
            invsb = pe_bcast_scalar(invs[:], 128, "invs")

            # own shard -> alphas [128, 8] via selection matmul
            attf2 = sm.tile([64, 128], FP32, tag="attf2")
            nc.gpsimd.dma_start(
                attf2[:], cc_att_out[:].rearrange("(k b) -> k b", k=64)
            )
            selsb = sm.tile([64, NC], FP32, tag="selsb")
            nc.gpsimd.dma_start(selsb[:], SEL)
            pa = pstr.tile([128, 8], FP32, tag="tr", name="pa_alpha")
            nc.tensor.matmul(pa[:], attf2[:], selsb[:], start=True, stop=True)
            exp_sh = sm.tile([128, 8], FP32, tag="expsh")
            nc.scalar.activation(exp_sh[:], pa[:], AF.Exp, bias=negmb[:])
            alphas = sm.tile([128, 8], FP32, tag="alphas")
            nc.vector.tensor_scalar_mul(alphas[:], exp_sh[:], invsb[:])

            # ---------- ctx = enc_shard.T @ alphas (PE native, 2nd enc pass) ----------
            ctx_row = sm.tile([1, C2H], FP32, tag="ctxrow")
            for half in range(2):
                prs = [
                    psrow.tile([1, 512], FP32, tag="rowp", name=f"ctxp{half}{cq}")
                    for cq in range(4)
                ]
                for b in range(8):
                    en2 = ws.tile([128, CH], FP32, tag="w", name="enc2t")
                    nc.sync.dma_start(
                        en2[:],
                        ENC[b * 128 : (b + 1) * 128, half * CH : (half + 1) * CH],
                    )
                    for cq in range(4):
                        nc.tensor.matmul(
                            prs[cq][:], alphas[:, b : b + 1],
                            en2[:, cq * 512 : (cq + 1) * 512],
                            start=(b == 0), stop=(b == 7),
                        )
                for cq in range(4):
                    nc.scalar.copy(
                        ctx_row[:, half * CH + cq * 512 : half * CH + (cq + 1) * 512],
                        prs[cq][:],
                    )
            cc_ctx_in = dram.tile([C2H], FP32, tag="ccctxi")
            cc_ctx_out = dram.tile([C2H], FP32, tag="ccctxo")
            nc.gpsimd.dma_start(cc_ctx_in[None, :], ctx_row[:])
            nc.gpsimd.collective_compute(
                "AllReduce", A.add, replica_groups=[list(range(NC))],
                ins=[cc_ctx_in[:].opt()], outs=[cc_ctx_out[:].opt()],
            )
            ctxb = bcast(load_row(cc_ctx_out[:], C2H, "ctx"), C2H, "ctx")

            # ---------- c = Wv @ ctx + bv + s_t ----------
            accc = sm.tile([128, 4], FP32, tag="accc")
            for half in range(2):
                for a in range(2):
                    wt = ws.tile([128, CH], FP32, tag="w", name="wv_t")
                    nc.sync.dma_start(
                        wt[:], WV[a * 128 : (a + 1) * 128, half * CH : (half + 1) * CH]
                    )
                    stt_gemv(
                        wt[:], ctxb[half][:],
                        accc[:, half * 2 + a : half * 2 + a + 1], f"c{half}{a}",
                    )
            cv0 = sm.tile([128, 2], FP32, tag="cv0")
            nc.vector.tensor_tensor(cv0[:], accc[:, :2], accc[:, 2:], A.add)
            cv = sm.tile([128, 2], FP32, tag="cv")
            nc.vector.tensor_tensor(cv[:], cv0[:], cinit[:], A.add)
            c_row_sh = to_row(cv[:], 2, "c")
            cc_c_in = dram.tile([SH], FP32, tag="ccci")
            cc_c_out = dram.tile([H], FP32, tag="ccco")
            nc.gpsimd.dma_start(cc_c_in[:].rearrange("(a b) -> a b", a=2), c_row_sh[:])
            nc.gpsimd.collective_compute(
                "AllGather", A.bypass, replica_groups=[list(range(NC))],
                ins=[cc_c_in[:].opt()], outs=[cc_c_out[:].opt()],
            )
            # ---------- GRU layers ----------
            tokb = bcast(load_row(TOK, H, "tok"), H, "tok")
            cb = bcast(load_row(cc_c_out[:], H, "c"), H, "c")
            xb_cur = [tokb[0], cb[0]]  # layer-0 input chunks [tok; c]

            cc_h_outs = []
            for l in range(L):
                hb = bcast(load_row(HID[l], H, f"h{l}"), H, f"h{l}")
                gacc = sm.tile([128, 18], FP32, tag="gacc", bufs=2, name=f"gacc{l}")
                # gh = W_hh[l] @ h_prev (chain-free)
                for g in range(3):
                    for a in range(2):
                        wt = ws.tile([128, CH], FP32, tag="w", name=f"whh{l}")
                        nc.sync.dma_start(wt[:], WHH[l, g, a * 128 : (a + 1) * 128, :])
                        stt_gemv(
                            wt[:], hb[0][:],
                            gacc[:, 12 + g * 2 + a : 13 + g * 2 + a], f"gh{l}{g}{a}",
                        )
                # gi = W_ih[l] @ x
                nchunks = 2 if l == 0 else 1
                for g in range(3):
                    for a in range(2):
                        for j in range(nchunks):
                            if l == 0:
                                wsrc = WIH0[g, a * 128 : (a + 1) * 128,
                                            j * CH : (j + 1) * CH]
                            else:
                                wsrc = WIHR[l - 1, g, a * 128 : (a + 1) * 128, :]
                            wt = ws.tile([128, CH], FP32, tag="w", name=f"wih{l}")
                            nc.sync.dma_start(wt[:], wsrc)
                            stt_gemv(
                                wt[:], xb_cur[j][:],
                                gacc[:, j * 6 + g * 2 + a : j * 6 + g * 2 + a + 1],
                                f"gi{l}{g}{a}{j}",
                            )
                giC = sm.tile([128, 6], FP32, tag="giC", bufs=2, name=f"giC{l}")
                if l == 0:
                    nc.vector.tensor_tensor(giC[:], gacc[:, :6], gacc[:, 6:12], A.add)
                else:
                    nc.vector.tensor_copy(giC[:], gacc[:, :6])
                bofs = l * 6  # (l*3+g)*2+a = l*6 + g*2 + a
                # r, z gates
                rz = []
                for gi_g in range(2):  # 0=r, 1=z
                    t1 = sm.tile([128, 2], FP32, tag="t1", bufs=2, name=f"t1_{l}{gi_g}")
                    nc.vector.tensor_tensor(
                        t1[:], giC[:, gi_g * 2 : gi_g * 2 + 2],
                        gacc[:, 12 + gi_g * 2 : 14 + gi_g * 2], A.add,
                    )
                    t2 = sm.tile([128, 2], FP32, tag="t2", bufs=2, name=f"t2_{l}{gi_g}")
                    nc.vector.tensor_tensor(
                        t2[:], t1[:], bsumT[:, bofs + gi_g * 2 : bofs + gi_g * 2 + 2],
                        A.add,
                    )
                    gt = sm.tile([128, 2], FP32, tag=f"g{gi_g}", bufs=2,
                                 name=f"gate{l}{gi_g}")
                    nc.scalar.activation(gt[:], t2[:], AF.Sigmoid)
                    rz.append(gt)
                r, z = rz
                # n gate
                gin = sm.tile([128, 2], FP32, tag="gin", bufs=2, name=f"gin{l}")
                nc.vector.tensor_tensor(
                    gin[:], giC[:, 4:6], bihT[:, bofs + 4 : bofs + 6], A.add
                )
                ghn = sm.tile([128, 2], FP32, tag="ghn", bufs=2, name=f"ghn{l}")
                nc.vector.tensor_tensor(
                    ghn[:], gacc[:, 16:18], bhhT[:, bofs + 4 : bofs + 6], A.add
                )
                rh = sm.tile([128, 2], FP32, tag="rh", bufs=2, name=f"rh{l}")
                nc.vector.tensor_tensor(rh[:], r[:], ghn[:], A.mult)
                ns = sm.tile([128, 2], FP32, tag="ns", bufs=2, name=f"ns{l}")
                nc.vector.tensor_tensor(ns[:], gin[:], rh[:], A.add)
                ng = sm.tile([128, 2], FP32, tag="ng", bufs=2, name=f"ng{l}")
                nc.scalar.activation(ng[:], ns[:], AF.Tanh)
                # h_new = (1-z)*n + z*h_prev
                zn = sm.tile([128, 2], FP32, tag="zn", bufs=2, name=f"zn{l}")
                nc.vector.tensor_tensor(zn[:], z[:], ng[:], A.mult)
                nm = sm.tile([128, 2], FP32, tag="nm", bufs=2, name=f"nm{l}")
                nc.vector.tensor_tensor(nm[:], ng[:], zn[:], A.subtract)
                zh = sm.tile([128, 2], FP32, tag="zh", bufs=2, name=f"zh{l}")
                nc.vector.tensor_tensor(zh[:], z[:], hpT[:, l * 2 : l * 2 + 2], A.mult)
                hn = sm.tile([128, 2], FP32, tag="hn", bufs=2, name=f"hn{l}")
                nc.vector.tensor_tensor(hn[:], nm[:], zh[:], A.add)
                hrow = to_row(hn[:], 2, f"hn{l}")
                cc_h_in = dram.tile([SH], FP32, tag=f"cchi{l}", name=f"cchi{l}")
                cc_h_out = dram.tile([H], FP32, tag=f"ccho{l}", name=f"ccho{l}")
                nc.gpsimd.dma_start(
                    cc_h_in[:].rearrange("(a b) -> a b", a=2), hrow[:]
                )
                nc.gpsimd.collective_compute(
                    "AllGather", A.bypass, replica_groups=[list(range(NC))],
                    ins=[cc_h_in[:].opt()], outs=[cc_h_out[:].opt()],
                )
                cc_h_outs.append(cc_h_out)
                if l < L - 1:
                    xb_cur = bcast(
                        load_row(cc_h_out[:], H, f"x{l + 1}"), H, f"x{l + 1}"
                    )

            # ---------- layernorm helper ----------
            def layer_norm(xv, n, width, g_t, b_t, nm):
                """xv: sbuf [n, 128] covering `width` elements; returns [n,128]."""
                d1 = sm.tile([n, 128], FP32, tag="lnd1", bufs=2, name=f"lnd1_{nm}")
                s_n = sm.tile([n, 1], FP32, tag="lns", bufs=2, name=f"lns_{nm}")
                nc.scalar.activation(d1[:], xv, AF.Copy, accum_out=s_n[:])
                ptot = pstr.tile([1, 1], FP32, tag="tr", name=f"lntot_{nm}")
                nc.tensor.matmul(ptot[:], s_n[:], onesp[:n, :], start=True, stop=True)
                negm_ = sm.tile([1, 1], FP32, tag="lnm", bufs=2, name=f"lnm_{nm}")
                nc.scalar.mul(negm_[:], ptot[:], -1.0 / width)
                negmn = pe_bcast_scalar(negm_[:], n, f"lnmb_{nm}")
                xc = sm.tile([n, 128], FP32, tag="lnxc", bufs=2, name=f"lnxc_{nm}")
                nc.scalar.activation(xc[:], xv, AF.Identity, bias=negmn[:])
                d2 = sm.tile([n, 128], FP32, tag="lnd2", bufs=2, name=f"lnd2_{nm}")
                q_n = sm.tile([n, 1], FP32, tag="lnq", bufs=2, name=f"lnq_{nm}")
                nc.scalar.activation(d2[:], xc[:], AF.Square, accum_out=q_n[:])
                ptot2 = pstr.tile([1, 1], FP32, tag="tr", name=f"lntot2_{nm}")
                nc.tensor.matmul(ptot2[:], q_n[:], onesp[:n, :], start=True, stop=True)
                vr = sm.tile([1, 1], FP32, tag="lnvr", bufs=2, name=f"lnvr_{nm}")
                nc.scalar.activation(
                    vr[:], ptot2[:], AF.Identity, bias=epsc[:], scale=1.0 / width
                )
                iv = sm.tile([1, 1], FP32, tag="lniv", bufs=2, name=f"lniv_{nm}")
                nc.vector.reciprocal(iv[:], vr[:])
                rstd = sm.tile([1, 1], FP32, tag="lnrs", bufs=2, name=f"lnrs_{nm}")
                nc.scalar.sqrt(rstd[:], iv[:])
                rstdn = pe_bcast_scalar(rstd[:], n, f"lnrsb_{nm}")
                xs = sm.tile([n, 128], FP32, tag="lnxs", bufs=2, name=f"lnxs_{nm}")
                nc.vector.tensor_scalar_mul(xs[:], xc[:], rstdn[:])
                xg = sm.tile([n, 128], FP32, tag="lnxg", bufs=2, name=f"lnxg_{nm}")
                nc.vector.tensor_tensor(xg[:], xs[:], g_t, A.mult)
                xo = sm.tile([n, 128], FP32, tag="lnxo", bufs=2, name=f"lnxo_{nm}")
                nc.vector.tensor_tensor(xo[:], xg[:], b_t, A.add)
                return xo

            # ---------- head: pre = [h4, tok, c]; LN; W1; gelu; W2 ----------
            got = sm.tile([48, 128], FP32, tag="got")
            nc.gpsimd.dma_start(got[:], GO.rearrange("(a b) -> a b", b=128))
            bot = sm.tile([48, 128], FP32, tag="bot")
            nc.gpsimd.dma_start(bot[:], BO.rearrange("(a b) -> a b", b=128))
            pv = sm.tile([48, 128], FP32, tag="pv")
            nc.gpsimd.dma_start(
                pv[:16, :], cc_h_outs[L - 1][:].rearrange("(a b) -> a b", a=16)
            )
            nc.gpsimd.dma_start(pv[16:32, :], TOK.rearrange("(a b) -> a b", a=16))
            nc.gpsimd.dma_start(
                pv[32:48, :], cc_c_out[:].rearrange("(a b) -> a b", a=16)
            )
            pln = layer_norm(pv[:], 48, 3 * H, got[:], bot[:], "pre")
            plnbuf = dram.tile([3 * H], FP32, tag="plnbuf")
            nc.gpsimd.dma_start(plnbuf[:].rearrange("(a b) -> a b", a=48), pln[:])
            plnb = bcast(load_row(plnbuf[:], 3 * H, "pln"), 3 * H, "pln")

            accp = sm.tile([128, 3], FP32, tag="accp")
            for j in range(3):
                wt = ws.tile([128, CH], FP32, tag="w", name="w1_t")
                nc.sync.dma_start(wt[:], W1[:, j * CH : (j + 1) * CH])
                stt_gemv(wt[:], plnb[j][:], accp[:, j : j + 1], f"p{j}")
            p01 = sm.tile([128, 1], FP32, tag="p01")
            nc.vector.tensor_tensor(p01[:], accp[:, 0:1], accp[:, 1:2], A.add)
            p012 = sm.tile([128, 1], FP32, tag="p012")
            nc.vector.tensor_tensor(p012[:], p01[:], accp[:, 2:3], A.add)
            pre2 = sm.tile([128, 1], FP32, tag="pre2")
            nc.vector.tensor_tensor(pre2[:], p012[:], b1T[:], A.add)
            # gelu(tanh approx)
            x2 = sm.tile([128, 1], FP32, tag="x2")
            nc.vector.tensor_tensor(x2[:], pre2[:], pre2[:], A.mult)
            x3 = sm.tile([128, 1], FP32, tag="x3")
            nc.vector.tensor_tensor(x3[:], x2[:], pre2[:], A.mult)
            tin = sm.tile([128, 1], FP32, tag="tin")
            nc.vector.scalar_tensor_tensor(
                out=tin[:], in0=x3[:], scalar=0.044715, in1=pre2[:],
                op0=A.mult, op1=A.add,
            )
            th = sm.tile([128, 1], FP32, tag="th")
            nc.scalar.activation(th[:], tin[:], AF.Tanh, scale=SQRT_2_OVER_PI)
            g1 = sm.tile([128, 1], FP32, tag="gelu1")
            nc.vector.scalar_tensor_tensor(
                out=g1[:], in0=th[:], scalar=1.0, in1=pre2[:],
                op0=A.add, op1=A.mult,
            )
            p2g = sm.tile([128, 1], FP32, tag="p2g")
            nc.vector.tensor_scalar_mul(p2g[:], g1[:], 0.5)
            p2row = to_row(p2g[:], 1, "p2")
            cc_p2_in = dram.tile([128], FP32, tag="ccp2i")
            cc_p2_out = dram.tile([H // 2], FP32, tag="ccp2o")
            nc.gpsimd.dma_start(cc_p2_in[None, :], p2row[:])
            nc.gpsimd.collective_compute(
                "AllGather", A.bypass, replica_groups=[list(range(NC))],
                ins=[cc_p2_in[:].opt()], outs=[cc_p2_out[:].opt()],
            )
            p2b = bcast(load_row(cc_p2_out[:], H // 2, "p2b"), H // 2, "p2b")

            w2sb = ws.tile([V, H // 2], FP32, tag="w", name="w2sb")
            nc.sync.dma_start(w2sb[:], W2)
            lgacc = sm.tile([V, 1], FP32, tag="lgacc")
            scr2 = scrp.tile([V, H // 2], FP32, tag="scr", name="scr2")
            nc.vector.scalar_tensor_tensor(
                out=scr2[:], in0=w2sb[:], scalar=1.0, in1=p2b[0][:V, : H // 2],
                op0=A.mult, op1=A.mult, accum_out=lgacc[:],
            )
            lg = sm.tile([V, 1], FP32, tag="lg")
            nc.vector.tensor_tensor(lg[:], lgacc[:], b2T[:], A.add)
            lgrow = to_row(lg[:], 1, "lg")  # -> [1, V]? transpose gives [1, 128]
            nc.gpsimd.dma_start(LOGITS[None, :], lgrow[:, :V])

            # ---------- h output layernorms ----------
            ght = sm.tile([16, 128], FP32, tag="ght")
            nc.gpsimd.dma_start(ght[:], GH.rearrange("(a b) -> a b", b=128))
            bht = sm.tile([16, 128], FP32, tag="bht")
            nc.gpsimd.dma_start(bht[:], BH.rearrange("(a b) -> a b", b=128))
            for l in range(L):
                hv = sm.tile([16, 128], FP32, tag="hv", bufs=2, name=f"hv{l}")
                nc.gpsimd.dma_start(
                    hv[:], cc_h_outs[l][:].rearrange("(a b) -> a b", a=16)
                )
                xo = layer_norm(hv[:], 16, H, ght[:], bht[:], f"h{l}")
                nc.gpsimd.dma_start(
                    HOUT[l, 0, :].rearrange("(a b) -> a b", a=16), xo[:]
                )

    nc.compile()
    return nc


def _to_row(col, n, nm):
    pass  # placeholder (helpers live inside _build)


def _get_nc():
    if "nc" not in _CACHE:
        _CACHE["nc"] = _build()
    return _CACHE["nc"]


def kernel(**inputs):
    from concourse import bass_utils

    f32 = lambda x: np.ascontiguousarray(np.asarray(x), dtype=np.float32)
    enc = f32(inputs["enc_output"])
    shot = int(np.asarray(inputs["shot"]).ravel()[0])
    hidden = f32(inputs["hidden"])      # [L, 1, H]
    emb = f32(inputs["emb"])
    Wq, bq = f32(inputs["Wq"]), f32(inputs["bq"])
    Wk, bk = f32(inputs["Wk"]), f32(inputs["bk"])  # bk: softmax-invariant shift
    Wv, bv = f32(inputs["Wv"]), f32(inputs["bv"])
    W_ih0 = f32(inputs["W_ih0"])
    W_ih_rest = f32(inputs["W_ih_rest"])
    W_hh = f32(inputs["W_hh"])
    b_ih, b_hh = f32(inputs["b_ih"]), f32(inputs["b_hh"])
    g_hid, beta_hid = f32(inputs["g_hid"]), f32(inputs["beta_hid"])
    g_out, beta_out = f32(inputs["g_out"]), f32(inputs["beta_out"])
    W1, b1 = f32(inputs["W1"]), f32(inputs["b1"])
    W2, b2 = f32(inputs["W2"]), f32(inputs["b2"])

    tok = emb[shot]
    hid2 = hidden[:, 0, :]

    def gate_shard(W3, i):
        # W3: [3H, K] -> [3, SH, K] per-gate rows for core i
        return np.stack([W3[g * H + i * SH : g * H + (i + 1) * SH] for g in range(3)])

    in_maps = []
    for i in range(NC):
        sel = np.zeros((64, NC), np.float32)
        for j in range(8):
            sel[8 * i + j, j] = 1.0
        m = {
            "enc": enc[i * TSH : (i + 1) * TSH],
            "wq": Wq[i * SH : (i + 1) * SH],
            "wk": Wk[i * SH : (i + 1) * SH],
            "wv": Wv[i * SH : (i + 1) * SH],
            "wih0": gate_shard(W_ih0, i),
            "wihr": np.stack([gate_shard(W_ih_rest[l], i) for l in range(L - 1)]),
            "whh": np.stack([gate_shard(W_hh[l], i) for l in range(L)]),
            "bih": np.stack(
                [[b_ih[l, g * H + i * SH : g * H + (i + 1) * SH] for g in range(3)]
                 for l in range(L)]
            ),
            "bhh": np.stack(
                [[b_hh[l, g * H + i * SH : g * H + (i + 1) * SH] for g in range(3)]
                 for l in range(L)]
            ),
            "bq": bq[i * SH : (i + 1) * SH],
            "bv": bv[i * SH : (i + 1) * SH],
            "w1": W1[i * 128 : (i + 1) * 128],
            "b1": b1[i * 128 : (i + 1) * 128],
            "w2": W2,
            "b2": b2,
            "hid": hid2,
            "hidsh": hid2[:, i * SH : (i + 1) * SH],
            "tok": tok,
            "g_hid": g_hid,
            "beta_hid": beta_hid,
            "g_out": g_out,
            "beta_out": beta_out,
            "selmat": sel,
        }
        in_maps.append({k: np.ascontiguousarray(v, np.float32) for k, v in m.items()})

    nc = _get_nc()
    res = bass_utils.run_bass_kernel_spmd(nc, in_maps, core_ids=list(range(NC)))
    _CACHE["last_results"] = res
    r0 = res.results[0]
    return (r0["logits"].copy(), r0["h_out"].copy())
